# revision 8
# baseline (speedup 1.0000x reference)
"""Trainium2 Bass kernel for nn_DiffeqSolver_KL.

Computes, elementwise over [64, 2048, 256] f32 tensors:
    K    = s + ln(-b' + c) - ln(s' + c)
    loss = EPS * b' * (K*S1 - S2)
where S1 = sum(a(m_t)), S2 = sum(a(m_t)*c(m_t)) are scalar time-sums over
t = 1..998 (computed host-side), c = 0.01, EPS = 0.001.

Rewritten for the hardware as (A = EPS*S1, BA = -S2/S1):
    t1  = Ln(-b' + c)          # ScalarE activation, scale=-1, bias=c
    t2  = Ln( s' + c)          # ScalarE activation, scale=+1, bias=c
    d   = t1 - t2              # VectorE tensor_tensor
    q   = (s + BA) + d         # VectorE scalar_tensor_tensor
    out = (q * A) * b'         # VectorE scalar_tensor_tensor
so loss = b'*(A*(s + t1 - t2) + A*BA) = EPS*b'*(K*S1 - S2).

b_phi_zt is not used by the reference computation and is never read.

Sharding: batch axis (64) split across 8 NeuronCores, 8 batches/core.
Per-core tensors are viewed as [128 partitions x 32768] f32 and streamed
through SBUF in [128 x 2048] tiles (1 MiB DMAs). Memory-bound: 64 MiB of
HBM traffic per core (3 loads + 1 store) ~ 180 us roofline.
"""

import os
import sys

import numpy as np

try:
    import concourse.bass as bass
except ImportError:  # harness may run without the repo on PYTHONPATH
    for _p in ("/opt/trn_rl_repo", "/root/.axon_site/_ro/trn_rl_repo"):
        if os.path.isdir(_p) and _p not in sys.path:
            sys.path.insert(0, _p)
    import concourse.bass as bass

import concourse.bacc as bacc
import concourse.mybir as mybir
import concourse.tile as tile
from concourse.bass_utils import run_bass_kernel_spmd

EPS = 0.001
C_CONST = 0.01
N_CORES = 8
BATCH, SEQ, DIM = 64, 2048, 256
PER_CORE_BATCH = BATCH // N_CORES
P = 128                                   # SBUF partitions
FREE = PER_CORE_BATCH * SEQ * DIM // P    # 32768
TILE_F = 2048


def _time_sums():
    t = np.arange(1, int(1.0 / EPS) - 1, dtype=np.float64)  # 1..998
    m = -1.0 + EPS * t
    a = -1.0 / (m * np.log(-m))
    c = np.log(-np.log(-m))
    return float(a.sum()), float((a * c).sum())


_S1, _S2 = _time_sums()
A_SCALE = float(np.float32(EPS * _S1))
BA_OFF = float(np.float32(-_S2 / _S1))

_nc = None


def _build(
    tile_f=TILE_F,
    io_bufs=3,
    tmp_bufs=2,
    inplace=False,
    store_engine="gpsimd",
    load_engines=("sync", "sync", "sync"),
    repeat=1,
):
    global _nc
    if _nc is not None and repeat == 1:
        return _nc
    nc = bacc.Bacc(
        "TRN2", target_bir_lowering=False, debug=False, num_devices=N_CORES
    )
    f32 = mybir.dt.float32
    bp_d = nc.dram_tensor("bp", [P, FREE], f32, kind="ExternalInput").ap()
    s_d = nc.dram_tensor("s", [P, FREE], f32, kind="ExternalInput").ap()
    sp_d = nc.dram_tensor("sp", [P, FREE], f32, kind="ExternalInput").ap()
    out_d = nc.dram_tensor("out", [P, FREE], f32, kind="ExternalOutput").ap()

    Ln = mybir.ActivationFunctionType.Ln
    add = mybir.AluOpType.add
    mult = mybir.AluOpType.mult
    n_tiles = FREE // tile_f

    def eng(name):
        return getattr(nc, name)

    with tile.TileContext(nc) as tc:
        with (
            tc.tile_pool(name="const", bufs=1) as const_pool,
            tc.tile_pool(name="io", bufs=io_bufs) as io_pool,
            tc.tile_pool(name="tmp", bufs=tmp_bufs) as tmp_pool,
        ):
            cbias = const_pool.tile([P, 1], f32)
            nc.gpsimd.memset(cbias[:], C_CONST)
            for i in range(n_tiles * repeat):
                i = i % n_tiles
                sl = bass.ts(i, tile_f)
                bp = io_pool.tile([P, tile_f], f32, tag="bp")
                eng(load_engines[0]).dma_start(bp[:], bp_d[:, sl])
                s = io_pool.tile([P, tile_f], f32, tag="s")
                eng(load_engines[1]).dma_start(s[:], s_d[:, sl])
                sp = io_pool.tile([P, tile_f], f32, tag="sp")
                eng(load_engines[2]).dma_start(sp[:], sp_d[:, sl])

                t1 = tmp_pool.tile([P, tile_f], f32, tag="t1")
                nc.scalar.activation(t1[:], bp[:], Ln, bias=cbias[:], scale=-1.0)
                if inplace:
                    t2, d, q, o = sp, t1, s, bp
                else:
                    t2 = tmp_pool.tile([P, tile_f], f32, tag="t2")
                    d = tmp_pool.tile([P, tile_f], f32, tag="d")
                    q = tmp_pool.tile([P, tile_f], f32, tag="q")
                    o = io_pool.tile([P, tile_f], f32, tag="o")
                nc.scalar.activation(t2[:], sp[:], Ln, bias=cbias[:], scale=1.0)
                nc.vector.tensor_sub(d[:], t1[:], t2[:])
                nc.vector.scalar_tensor_tensor(q[:], s[:], BA_OFF, d[:], add, add)
                nc.vector.scalar_tensor_tensor(o[:], q[:], A_SCALE, bp[:], mult, mult)

                eng(store_engine).dma_start(out_d[:, sl], o[:])

    nc.compile()
    if repeat == 1:
        _nc = nc
    return nc


def _in_maps(bd, st, sd):
    maps = []
    for c in range(N_CORES):
        sl = slice(c * PER_CORE_BATCH, (c + 1) * PER_CORE_BATCH)
        maps.append(
            {
                "bp": np.ascontiguousarray(bd[sl]).reshape(P, FREE),
                "s": np.ascontiguousarray(st[sl]).reshape(P, FREE),
                "sp": np.ascontiguousarray(sd[sl]).reshape(P, FREE),
            }
        )
    return maps


def kernel(
    b_phi_zt=None, b_phi_zt_deriv=None, s_phi_zt=None, s_phi_zt_deriv=None
):
    nc = _build()
    bd = np.asarray(b_phi_zt_deriv, dtype=np.float32)
    st = np.asarray(s_phi_zt, dtype=np.float32)
    sd = np.asarray(s_phi_zt_deriv, dtype=np.float32)
    res = run_bass_kernel_spmd(nc, _in_maps(bd, st, sd), list(range(N_CORES)))
    out = np.empty((BATCH, SEQ, DIM), dtype=np.float32)
    for c in range(N_CORES):
        out[c * PER_CORE_BATCH : (c + 1) * PER_CORE_BATCH] = res.results[c][
            "out"
        ].reshape(PER_CORE_BATCH, SEQ, DIM)
    return out


# revision 14
# speedup vs baseline: 1.7427x; 1.7427x over previous
"""Trainium2 Bass kernel for nn_DiffeqSolver_KL.

Computes, elementwise over [64, 2048, 256] f32 tensors:
    K    = s + ln(-b' + c) - ln(s' + c)
    loss = EPS * b' * (K*S1 - S2)
where S1 = sum(a(m_t)), S2 = sum(a(m_t)*c(m_t)) are scalar time-sums over
t = 1..998 (computed host-side), c = 0.01, EPS = 0.001.

Rewritten for the hardware as (A = EPS*S1, BA = -S2/S1):
    t1  = Ln(-b' + c)          # ScalarE activation, scale=-1, bias=c
    t2  = Ln( s' + c)          # ScalarE activation, scale=+1, bias=c
    d   = t1 - t2              # VectorE tensor_tensor
    q   = (s + BA) + d         # VectorE scalar_tensor_tensor
    out = (q * A) * b'         # VectorE scalar_tensor_tensor
so loss = b'*(A*(s + t1 - t2) + A*BA) = EPS*b'*(K*S1 - S2).

b_phi_zt is not used by the reference computation and is never read.

Sharding: batch axis (64) split across 8 NeuronCores, 8 batches/core.
Per-core tensors are viewed as [128 partitions x 32768] f32 and streamed
through SBUF in [128 x 2048] tiles (1 MiB DMAs). Memory-bound: 64 MiB of
HBM traffic per core (3 loads + 1 store) ~ 180 us roofline.
"""

import os
import sys

import numpy as np

try:
    import concourse.bass as bass
except ImportError:  # harness may run without the repo on PYTHONPATH
    for _p in ("/opt/trn_rl_repo", "/root/.axon_site/_ro/trn_rl_repo"):
        if os.path.isdir(_p) and _p not in sys.path:
            sys.path.insert(0, _p)
    import concourse.bass as bass

import concourse.bacc as bacc
import concourse.mybir as mybir
import concourse.tile as tile
from concourse.bass_utils import run_bass_kernel_spmd

EPS = 0.001
C_CONST = 0.01
N_CORES = 8
BATCH, SEQ, DIM = 64, 2048, 256
PER_CORE_BATCH = BATCH // N_CORES
P = 128                                   # SBUF partitions
FREE = PER_CORE_BATCH * SEQ * DIM // P    # 32768
TILE_F = 2048


def _time_sums():
    t = np.arange(1, int(1.0 / EPS) - 1, dtype=np.float64)  # 1..998
    m = -1.0 + EPS * t
    a = -1.0 / (m * np.log(-m))
    c = np.log(-np.log(-m))
    return float(a.sum()), float((a * c).sum())


_S1, _S2 = _time_sums()
A_SCALE = float(np.float32(EPS * _S1))
BA_OFF = float(np.float32(-_S2 / _S1))

_nc = None


def _build(
    tile_f=TILE_F,
    io_bufs=3,
    tmp_bufs=2,
    inplace=False,
    store_engine="gpsimd",
    load_engines=("sync", "sync", "sync"),
    repeat=1,
    contig=False,
):
    global _nc
    if _nc is not None and repeat == 1:
        return _nc
    nc = bacc.Bacc(
        "TRN2", target_bir_lowering=False, debug=False, num_devices=N_CORES
    )
    f32 = mybir.dt.float32
    n_tiles_decl = FREE // tile_f
    if contig:
        # each [P, tile_f] tile is one contiguous DRAM span
        dshape = [n_tiles_decl, P, tile_f]
    else:
        dshape = [P, FREE]
    bp_d = nc.dram_tensor("bp", dshape, f32, kind="ExternalInput").ap()
    s_d = nc.dram_tensor("s", dshape, f32, kind="ExternalInput").ap()
    sp_d = nc.dram_tensor("sp", dshape, f32, kind="ExternalInput").ap()
    out_d = nc.dram_tensor("out", dshape, f32, kind="ExternalOutput").ap()

    Ln = mybir.ActivationFunctionType.Ln
    add = mybir.AluOpType.add
    mult = mybir.AluOpType.mult
    n_tiles = FREE // tile_f

    def eng(name):
        return getattr(nc, name)

    with tile.TileContext(nc) as tc:
        with (
            tc.tile_pool(name="const", bufs=1) as const_pool,
            tc.tile_pool(name="io", bufs=io_bufs) as io_pool,
            tc.tile_pool(name="tmp", bufs=tmp_bufs) as tmp_pool,
        ):
            cbias = const_pool.tile([P, 1], f32)
            nc.gpsimd.memset(cbias[:], C_CONST)
            for i in range(n_tiles * repeat):
                i = i % n_tiles
                if contig:
                    bp_src, s_src, sp_src = bp_d[i], s_d[i], sp_d[i]
                    out_dst = out_d[i]
                else:
                    sl = bass.ts(i, tile_f)
                    bp_src, s_src, sp_src = bp_d[:, sl], s_d[:, sl], sp_d[:, sl]
                    out_dst = out_d[:, sl]
                bp = io_pool.tile([P, tile_f], f32, tag="bp")
                eng(load_engines[0]).dma_start(bp[:], bp_src)
                s = io_pool.tile([P, tile_f], f32, tag="s")
                eng(load_engines[1]).dma_start(s[:], s_src)
                sp = io_pool.tile([P, tile_f], f32, tag="sp")
                eng(load_engines[2]).dma_start(sp[:], sp_src)

                t1 = tmp_pool.tile([P, tile_f], f32, tag="t1")
                nc.scalar.activation(t1[:], bp[:], Ln, bias=cbias[:], scale=-1.0)
                if inplace:
                    t2, d, q, o = sp, t1, s, bp
                else:
                    t2 = tmp_pool.tile([P, tile_f], f32, tag="t2")
                    d = tmp_pool.tile([P, tile_f], f32, tag="d")
                    q = tmp_pool.tile([P, tile_f], f32, tag="q")
                    o = io_pool.tile([P, tile_f], f32, tag="o")
                nc.scalar.activation(t2[:], sp[:], Ln, bias=cbias[:], scale=1.0)
                nc.vector.tensor_sub(d[:], t1[:], t2[:])
                nc.vector.scalar_tensor_tensor(q[:], s[:], BA_OFF, d[:], add, add)
                nc.vector.scalar_tensor_tensor(o[:], q[:], A_SCALE, bp[:], mult, mult)

                eng(store_engine).dma_start(out_dst, o[:])

    nc._dshape = tuple(dshape)
    nc.compile()
    if repeat == 1:
        _nc = nc
    return nc


def _in_maps(bd, st, sd, dshape=(P, FREE)):
    maps = []
    for c in range(N_CORES):
        sl = slice(c * PER_CORE_BATCH, (c + 1) * PER_CORE_BATCH)
        maps.append(
            {
                "bp": np.ascontiguousarray(bd[sl]).reshape(dshape),
                "s": np.ascontiguousarray(st[sl]).reshape(dshape),
                "sp": np.ascontiguousarray(sd[sl]).reshape(dshape),
            }
        )
    return maps


def kernel(
    b_phi_zt=None, b_phi_zt_deriv=None, s_phi_zt=None, s_phi_zt_deriv=None
):
    nc = _build()
    bd = np.asarray(b_phi_zt_deriv, dtype=np.float32)
    st = np.asarray(s_phi_zt, dtype=np.float32)
    sd = np.asarray(s_phi_zt_deriv, dtype=np.float32)
    maps = _in_maps(bd, st, sd, dshape=nc._dshape)
    res = run_bass_kernel_spmd(nc, maps, list(range(N_CORES)))
    out = np.empty((BATCH, SEQ, DIM), dtype=np.float32)
    for c in range(N_CORES):
        out[c * PER_CORE_BATCH : (c + 1) * PER_CORE_BATCH] = res.results[c][
            "out"
        ].reshape(PER_CORE_BATCH, SEQ, DIM)
    return out
